# revision 1
# baseline (speedup 1.0000x reference)
"""GroupedQueryAttention Trainium2 kernel (8-core SPMD).

Problem: B=2, S=2048, D=2048, 32 Q heads, 8 KV groups, head_dim=64.
  q = xq @ Wq + bq; k = xk @ Wk + bk; v = xv @ Wv + bv
  logits = q . k / sqrt(512), causal softmax, out = (attn @ v) @ Wo + bo

Sharding: one batch x two KV groups per core (2 batches x 4 group-pairs = 8).
Each core computes its 8 Q heads' attention and a partial output projection
(rows of Wo for its 512 channels); the host sums the 4 partials per batch and
adds the bv/bo corrections (bv passes through softmax linearly since attention
weights sum to 1, so bv_expand @ Wo + bo is exact).

Device layout notes:
- All matmuls contract over the SBUF partition dim. The host passes x^T
  (k-major) so projections need no on-device transpose.
- Logits are computed transposed (lT[n, m]) so that attn@v needs no transpose
  either: outT[d, m] = sum_n v[n, d] * pT[n, m], and outT[c, m] is exactly the
  lhsT the Wo matmul wants.
- Q/K heads for the two groups are stacked in partition halves (host permutes
  Wq columns / Wo rows pair-major), so one 128-partition LDWEIGHTS serves two
  row-tiled (K=64) logits matmuls and two col-tiled (M=64) attn matmuls.
- Softmax denominators come from ones-vector matmuls (partition reduction on
  the PE), reciprocals on DVE, then a PE outer-product broadcast (ones-col x
  recip-row) feeds one fused normalize+evict multiply per pair.
- Causal masking: n-block x m-superblock tiles with n > m skipped entirely,
  diagonal tiles width-trimmed, triangle handled by one constant upper-tri
  mask multiply.
"""

import math
import numpy as np

import concourse.bass as bass
import concourse.mybir as mybir
from concourse import tile
from concourse.bass_utils import run_bass_kernel_spmd
from concourse.vector_clock import ScopedClock

F32 = mybir.dt.float32
B, S, D = 2, 2048, 2048
NKV, HPG, HD = 8, 4, 64
DIMK = 512                 # k/v projection width; also the softmax scale base
CPC = 512                  # q channels per core (2 groups * 4 heads * 64)
KC = D // 128              # 16 k-chunks
MSB = S // 512             # 4 m-superblocks
NB = S // 128              # 16 n-blocks
INV_SQRT_DIMK = 1.0 / math.sqrt(float(DIMK))


# ---------------------------------------------------------------------------
# TileContext tail-drain patch: the bundled neuronxcc walrus rejects
# instructions carrying more than ~2 sync waits ("Too many sync wait
# commands"). Spread the kernel-tail waits over single-wait nops.
def _patched_drain_and_barrier(self, tick_clock, wait_clock):
    nc = self.nc
    collector = nc.sync.nop(nofuse=True)
    wait_clock.add_sem_waits(
        collector.ins, ScopedClock({None: tick_clock.global_clock})
    )
    si = collector.ins.sync_info
    waits = list(si.on_wait) if si is not None and si.on_wait else []
    if waits:
        collector.ins.sync_info = mybir.SyncInfo(
            on_wait=[waits[0]], on_update=list(si.on_update or [])
        )
        for w in waits[1:]:
            extra = nc.sync.nop(nofuse=True)
            extra.ins.sync_info = mybir.SyncInfo(on_wait=[w], on_update=[])
    nc.sync.drain()
    nc.all_engine_barrier()
    assert self.sems is not None
    popped = nc._tile_sem_poison_stack.pop()
    assert popped is self._sem_poison
    nc.clear_and_free_semaphores(list(self.sems.allocated().values()))
    nc.all_engine_barrier()


tile.TileContext._drain_and_barrier = _patched_drain_and_barrier


_MAXW = 1
_NOPID = [0]


def split_excess_waits(nc):
    """Walrus here encodes at most ~1-2 sync waits per instruction; move the
    excess onto preceding same-engine nops (engine order preserves timing)."""
    for f in nc.m.functions:
        for bb in f.blocks:
            out_list = []
            changed = False
            for inst in bb.instructions:
                si = getattr(inst, "sync_info", None)
                waits = list(si.on_wait) if si is not None and si.on_wait else []
                if len(waits) > _MAXW:
                    changed = True
                    for w in waits[:-_MAXW]:
                        _NOPID[0] += 1
                        nop = mybir.InstNoOp(
                            name=f"waitnop-{_NOPID[0]}", ins=[], outs=[],
                            engine=inst.engine,
                        )
                        nop.sync_info = mybir.SyncInfo(on_wait=[w], on_update=[])
                        out_list.append(nop)
                    inst.sync_info = mybir.SyncInfo(
                        on_wait=waits[-_MAXW:], on_update=list(si.on_update or [])
                    )
                out_list.append(inst)
            if changed:
                bb.instructions[:] = out_list
# ---------------------------------------------------------------------------


def build_bass():
    nc = bass.Bass()
    xqT = nc.dram_tensor("xqT", [D, S], F32, kind="ExternalInput")
    xkT = nc.dram_tensor("xkT", [D, S], F32, kind="ExternalInput")
    xvT = nc.dram_tensor("xvT", [D, S], F32, kind="ExternalInput")
    wq = nc.dram_tensor("wq", [D, CPC], F32, kind="ExternalInput")
    wk = nc.dram_tensor("wk", [D, 128], F32, kind="ExternalInput")
    wv = nc.dram_tensor("wv", [D, 128], F32, kind="ExternalInput")
    wo = nc.dram_tensor("wo", [CPC, D], F32, kind="ExternalInput")
    bq = nc.dram_tensor("bq", [CPC, 1], F32, kind="ExternalInput")
    bk = nc.dram_tensor("bk", [128, 1], F32, kind="ExternalInput")
    trimask = nc.dram_tensor("trimask", [128, 128], F32, kind="ExternalInput")
    out = nc.dram_tensor("out", [S, D], F32, kind="ExternalOutput")

    from contextlib import ExitStack
    with tile.TileContext(nc) as tc, ExitStack() as ctx:
        build_body(ctx, tc, xqT, xkT, xvT, wq, wk, wv, wo, bq, bk, trimask, out)
    split_excess_waits(nc)
    return nc


def build_body(ctx, tc, xqT, xkT, xvT, wq, wk, wv, wo, bq, bk, trimask, out):
    nc = tc.nc
    Exp = mybir.ActivationFunctionType.Exp
    Ident = mybir.ActivationFunctionType.Identity

    const = ctx.enter_context(tc.tile_pool(name="const", bufs=1))
    wq_sb = const.tile([128, KC * CPC // 128 * 128], F32, tag="wq")  # [128, 8192]
    wk_sb = const.tile([128, KC * 128], F32, tag="wk")               # [128, 2048]
    wv_sb = const.tile([128, KC * 128], F32, tag="wv")               # [128, 2048]
    wo_sb = const.tile([128, 4 * D], F32, tag="wo")                  # [128, 8192]
    kT_sb = const.tile([128, S], F32, tag="kT")                      # [128, 2048]
    v_sb = const.tile([128, S], F32, tag="v")                        # [128, 2048]
    qT_sb = const.tile([128, 4 * S], F32, tag="qT")                  # [128, 8192]
    bq_sb = const.tile([128, 4], F32, tag="bq")
    bk_sb = const.tile([128, 1], F32, tag="bk")
    mask_sb = const.tile([128, 128], F32, tag="mask")
    ones_sb = const.tile([128, 1], F32, tag="ones")
    ones_row = const.tile([1, 64], F32, tag="ones_row")

    # Weight / bias / mask loads
    nc.sync.dma_start(
        wq_sb[:].rearrange("p (kc c) -> p kc c", kc=KC),
        wq.rearrange("(kc p) c -> p kc c", p=128),
    )
    nc.sync.dma_start(
        wk_sb[:].rearrange("p (kc c) -> p kc c", kc=KC),
        wk.rearrange("(kc p) c -> p kc c", p=128),
    )
    nc.sync.dma_start(
        wv_sb[:].rearrange("p (kc c) -> p kc c", kc=KC),
        wv.rearrange("(kc p) c -> p kc c", p=128),
    )
    nc.sync.dma_start(
        wo_sb[:].rearrange("p (cb d) -> p cb d", cb=4),
        wo.rearrange("(cb p) d -> p cb d", p=128),
    )
    nc.sync.dma_start(
        bq_sb[:].rearrange("p (cb o) -> p cb o", cb=4),
        bq.rearrange("(cb p) o -> p cb o", p=128),
    )
    nc.sync.dma_start(bk_sb[:], bk[:])
    nc.sync.dma_start(mask_sb[:], trimask[:])
    nc.vector.memset(ones_sb[:], 1.0)
    nc.vector.memset(ones_row[:], 1.0)

    # ---------------- Phase 1-3: projections (4 psum banks) ----------------
    with tc.tile_pool(name="proj_psum", bufs=4, space="PSUM") as proj_psum, \
         tc.tile_pool(name="xin", bufs=3) as xin_pool, \
         tc.tile_pool(name="xvin", bufs=3) as xvin_pool:

        # K projection: kT[c=128, n] accumulated over k-chunks, bias bk.
        for nsb in range(4):
            ps = proj_psum.tile([128, 512], F32, tag="ps")
            for kc in range(KC):
                xk_t = xin_pool.tile([128, 512], F32, tag="xk")
                nc.scalar.dma_start(
                    xk_t[:], xkT[kc * 128:(kc + 1) * 128, nsb * 512:(nsb + 1) * 512]
                )
                nc.tensor.matmul(
                    ps[:], wk_sb[:, kc * 128:(kc + 1) * 128], xk_t[:],
                    start=(kc == 0), stop=(kc == KC - 1),
                )
            nc.scalar.activation(
                kT_sb[:, nsb * 512:(nsb + 1) * 512], ps[:], Ident, bias=bk_sb[:]
            )

        # V projection: v[n=128, c=128] natural layout, two n-blocks per pass.
        for nbp in range(8):
            psa = proj_psum.tile([128, 128], F32, tag="ps")
            psb = proj_psum.tile([128, 128], F32, tag="ps")
            for kc in range(KC):
                xv_t = xvin_pool.tile([128, 256], F32, tag="xv")
                nc.scalar.dma_start(
                    xv_t[:], xvT[kc * 128:(kc + 1) * 128, nbp * 256:(nbp + 1) * 256]
                )
                nc.tensor.matmul(
                    psa[:], xv_t[:, 0:128],
                    wv_sb[:, kc * 128:(kc + 1) * 128],
                    start=(kc == 0), stop=(kc == KC - 1),
                )
                nc.tensor.matmul(
                    psb[:], xv_t[:, 128:256],
                    wv_sb[:, kc * 128:(kc + 1) * 128],
                    start=(kc == 0), stop=(kc == KC - 1),
                )
            nc.vector.tensor_copy(v_sb[:, (2 * nbp) * 128:(2 * nbp + 1) * 128], psa[:])
            nc.vector.tensor_copy(v_sb[:, (2 * nbp + 1) * 128:(2 * nbp + 2) * 128], psb[:])

        # Q projection: qT[c, m] for all four channel blocks, bias bq.
        for msb in range(MSB):
            pss = [proj_psum.tile([128, 512], F32, tag="ps", name=f"psq{cb}")
                   for cb in range(4)]
            for kc in range(KC):
                xq_t = xin_pool.tile([128, 512], F32, tag="xk")
                nc.scalar.dma_start(
                    xq_t[:], xqT[kc * 128:(kc + 1) * 128, msb * 512:(msb + 1) * 512]
                )
                for cb in range(4):
                    nc.tensor.matmul(
                        pss[cb][:],
                        wq_sb[:, kc * CPC + cb * 128: kc * CPC + (cb + 1) * 128],
                        xq_t[:],
                        start=(kc == 0), stop=(kc == KC - 1),
                    )
            for cb in range(4):
                nc.scalar.activation(
                    qT_sb[:, cb * S + msb * 512: cb * S + (msb + 1) * 512],
                    pss[cb][:], Ident, bias=bq_sb[:, cb:cb + 1],
                )

    # ---------------- Phase 4: attention + output projection ----------------
    with tc.tile_pool(name="lt_psum", bufs=2, space="PSUM") as lt_psum, \
         tc.tile_pool(name="acc_psum", bufs=2, space="PSUM") as acc_psum, \
         tc.tile_pool(name="den_psum", bufs=2, space="PSUM") as den_psum, \
         tc.tile_pool(name="wo_psum", bufs=2, space="PSUM") as wo_psum, \
         tc.tile_pool(name="pt", bufs=6) as pt_pool, \
         tc.tile_pool(name="outT", bufs=2) as outT_pool, \
         tc.tile_pool(name="nrm", bufs=3) as nrm_pool, \
         tc.tile_pool(name="osb", bufs=3) as out_pool:

        for msb in range(MSB):
            outT_t = outT_pool.tile([128, 2048], F32, tag="outT")
            for p in range(4):
                acc = acc_psum.tile([128, 512], F32, tag="acc")
                den_a = den_psum.tile([128, 512], F32, tag="den", name=f"dena{msb}{p}")
                den_b = den_psum.tile([128, 512], F32, tag="den", name=f"denb{msb}{p}")
                njb = 4 * msb + 4
                for j in range(njb):
                    if j < 4 * msb:
                        moff, W = 0, 512
                    else:
                        t = j - 4 * msb
                        moff, W = 128 * t, 512 - 128 * t
                    first = (j == 0)
                    last = (j == njb - 1)
                    qlo = qT_sb[0:64, p * S + msb * 512 + moff:
                                p * S + msb * 512 + moff + W]
                    qhi = qT_sb[64:128, p * S + msb * 512 + moff:
                                p * S + msb * 512 + moff + W]
                    l0 = lt_psum.tile([128, 512], F32, tag="lt")
                    l1 = lt_psum.tile([128, 512], F32, tag="lt")
                    nc.tensor.matmul(
                        l0[:, 0:W],
                        kT_sb[0:64, j * 128:(j + 1) * 128], qlo,
                        start=True, stop=True, tile_position=(0, 0),
                    )
                    nc.tensor.matmul(
                        l1[:, 0:W],
                        kT_sb[64:128, j * 128:(j + 1) * 128], qhi,
                        start=True, stop=True, tile_position=(64, 0),
                    )
                    p0 = pt_pool.tile([128, 512], F32, tag="pt")
                    p1 = pt_pool.tile([128, 512], F32, tag="pt")
                    nc.scalar.activation(p0[:, 0:W], l0[:, 0:W], Exp,
                                         scale=INV_SQRT_DIMK)
                    nc.scalar.activation(p1[:, 0:W], l1[:, 0:W], Exp,
                                         scale=INV_SQRT_DIMK)
                    if j >= 4 * msb:  # diagonal: mask the leading triangle
                        nc.vector.tensor_mul(p0[:, 0:128], p0[:, 0:128], mask_sb[:])
                        nc.vector.tensor_mul(p1[:, 0:128], p1[:, 0:128], mask_sb[:])
                    # attn @ v, col-tiled pair into one accumulating bank
                    nc.tensor.matmul(
                        acc[0:64, moff:moff + W],
                        v_sb[:, j * 128: j * 128 + 64], p0[:, 0:W],
                        start=first, stop=last, tile_position=(0, 0),
                    )
                    nc.tensor.matmul(
                        acc[64:128, moff:moff + W],
                        v_sb[:, j * 128 + 64: j * 128 + 128], p1[:, 0:W],
                        start=first, stop=last, tile_position=(0, 64),
                    )
                    # denominators via ones-vector partition reduction
                    nc.tensor.matmul(
                        den_a[0:1, moff:moff + W],
                        ones_sb[:], p0[:, 0:W],
                        start=first, stop=last, tile_position=(0, 0),
                    )
                    nc.tensor.matmul(
                        den_b[32:33, moff:moff + W],
                        ones_sb[:], p1[:, 0:W],
                        start=first, stop=last, tile_position=(0, 32),
                    )
                # normalize: reciprocal rows -> PE outer-product broadcast
                # (ones-col x recip-row) -> fused multiply during psum eviction
                recip_a = nrm_pool.tile([1, 512], F32, tag="recipa",
                                        name=f"rca{msb}{p}")
                recip_b = nrm_pool.tile([1, 512], F32, tag="recipb",
                                        name=f"rcb{msb}{p}")
                bcast_ps = den_psum.tile([128, 512], F32, tag="den",
                                        name=f"bcp{msb}{p}")
                bcast = nrm_pool.tile([128, 512], F32, tag="bcast")
                nc.vector.reciprocal(recip_a[:], den_a[0:1, :])
                nc.vector.reciprocal(recip_b[:], den_b[32:33, :])
                nc.tensor.matmul(
                    bcast_ps[0:64, :], ones_row[:], recip_a[:],
                    start=True, stop=True, tile_position=(0, 0),
                )
                nc.tensor.matmul(
                    bcast_ps[64:128, :], ones_row[:], recip_b[:],
                    start=True, stop=True, tile_position=(0, 64),
                )
                nc.vector.tensor_copy(bcast[:], bcast_ps[:])
                nc.vector.tensor_mul(
                    outT_t[:, p * 512:(p + 1) * 512], acc[:], bcast[:]
                )

            # Output projection for this m-superblock
            for mb in range(4):
                for db in range(4):
                    pso = wo_psum.tile([128, 512], F32, tag="wo")
                    for cb in range(4):
                        nc.tensor.matmul(
                            pso[:],
                            outT_t[:, cb * 512 + mb * 128: cb * 512 + (mb + 1) * 128],
                            wo_sb[:, cb * D + db * 512: cb * D + (db + 1) * 512],
                            start=(cb == 0), stop=(cb == 3),
                        )
                    o_t = out_pool.tile([128, 512], F32, tag="osb")
                    nc.vector.tensor_copy(o_t[:], pso[:])
                    nc.sync.dma_start(
                        out[msb * 512 + mb * 128: msb * 512 + (mb + 1) * 128,
                            db * 512:(db + 1) * 512],
                        o_t[:],
                    )


_NC_CACHE = {}


def get_nc():
    if "nc" not in _NC_CACHE:
        _NC_CACHE["nc"] = build_bass()
    return _NC_CACHE["nc"]


def kernel(inputs_q, inputs_k, inputs_v, Wq, bq, Wk, bk, Wv, bv, Wo, bo):
    inputs_q = np.asarray(inputs_q, np.float32)
    inputs_k = np.asarray(inputs_k, np.float32)
    inputs_v = np.asarray(inputs_v, np.float32)
    Wq = np.asarray(Wq, np.float32)
    Wk = np.asarray(Wk, np.float32)
    Wv = np.asarray(Wv, np.float32)
    Wo = np.asarray(Wo, np.float32)
    bq = np.asarray(bq, np.float32)
    bk = np.asarray(bk, np.float32)
    bv = np.asarray(bv, np.float32)
    bo = np.asarray(bo, np.float32)

    nc = get_nc()
    trimask = np.triu(np.ones((128, 128), np.float32))  # mask[n, m] = m >= n

    xT = {}
    for b in range(B):
        xT[("q", b)] = np.ascontiguousarray(inputs_q[b].T)
        xT[("k", b)] = np.ascontiguousarray(inputs_k[b].T)
        xT[("v", b)] = np.ascontiguousarray(inputs_v[b].T)

    in_maps = []
    perms = []
    for c in range(8):
        b = c // 4
        g0 = 2 * (c % 4)
        g1 = g0 + 1
        # pair-major channel permutation: (head p of g0, head p of g1), p=0..3
        perm = []
        for p in range(HPG):
            perm.extend(range(256 * g0 + 64 * p, 256 * g0 + 64 * p + 64))
            perm.extend(range(256 * g1 + 64 * p, 256 * g1 + 64 * p + 64))
        perm = np.array(perm)
        perms.append(perm)
        in_maps.append({
            "xqT": xT[("q", b)],
            "xkT": xT[("k", b)],
            "xvT": xT[("v", b)],
            "wq": np.ascontiguousarray(Wq[:, perm]),
            "wk": np.ascontiguousarray(Wk[:, 64 * g0: 64 * g0 + 128]),
            "wv": np.ascontiguousarray(Wv[:, 64 * g0: 64 * g0 + 128]),
            "wo": np.ascontiguousarray(Wo[perm, :]),
            "bq": np.ascontiguousarray(bq[perm].reshape(CPC, 1)),
            "bk": np.ascontiguousarray(bk[64 * g0: 64 * g0 + 128].reshape(128, 1)),
            "trimask": trimask,
        })

    res = run_bass_kernel_spmd(nc, in_maps, list(range(8)))

    # bv passes through (attention rows sum to 1): out += bv_expand @ Wo + bo
    bv_expand = np.repeat(bv.reshape(NKV, 1, HD), HPG, axis=1).reshape(D)
    corr = (bv_expand.astype(np.float64) @ Wo.astype(np.float64)) + bo

    outp = np.zeros((B, S, D), np.float64)
    for c in range(8):
        outp[c // 4] += res.results[c]["out"].astype(np.float64)
    outp += corr
    return outp.astype(np.float32)



# revision 5
# speedup vs baseline: 2.3787x; 2.3787x over previous
"""GroupedQueryAttention Trainium2 kernel (8-core SPMD, bf16 datapath).

Problem: B=2, S=2048, D=2048, 32 Q heads, 8 KV groups, head_dim=64.
  q = xq @ Wq + bq; k = xk @ Wk + bk; v = xv @ Wv + bv
  logits = q . k / sqrt(512), causal softmax, out = (attn @ v) @ Wo + bo

Sharding: one batch x two KV groups per core (2 batches x 4 group-pairs = 8).
Each core computes its 8 Q heads' attention and a partial output projection
(rows of Wo for its 512 channels); the host sums the 4 partials per batch and
adds the bv/bo corrections (bv passes through softmax linearly since attention
weights sum to 1, so bv_expand @ Wo + bo is exact).

Perf notes vs the fp32 version:
- All matmul operands are bf16 (host casts inputs/weights; PSUM accumulation
  stays fp32): 1 PE cycle/row at any width vs fp32's two-pass LOW_HIGH mode.
- Softmax denominators are folded into the attn@v matmul via a ones column
  appended to each head-group's V block (M=65), killing the dedicated
  ones-vector matmul streams.
- V is projected transposed (weights stationary, x streaming) then flipped
  with PE transpose ops - much cheaper than streaming 128-wide W with x tiles
  as stationary weights.
- The two logit halves of a key block land in one 2-bank PSUM tile so a single
  wide activation does exp for both (fewer Act fixed overheads).
- Reciprocal uses the fast-approx DVE op (f32), downcast to bf16 on the Act
  engine (single-partition DVE ops are lane-serial and slow).
- Input/output DMA spread across scalar/gpsimd/sync queues (~95GB/s each).
- Wo projection of superblock i is emitted after the first head-pair of
  superblock i+1 so its matmuls never head-block the PE queue.
"""

import math
import numpy as np
import ml_dtypes

import concourse.bass as bass
import concourse.mybir as mybir
from concourse import tile
from concourse.bass_utils import run_bass_kernel_spmd
from concourse.vector_clock import ScopedClock

F32 = mybir.dt.float32
BF16 = mybir.dt.bfloat16
NPBF16 = ml_dtypes.bfloat16
B, S, D = 2, 2048, 2048
NKV, HPG, HD = 8, 4, 64
DIMK = 512                 # k/v projection width; also the softmax scale base
CPC = 512                  # q channels per core (2 groups * 4 heads * 64)
KC = D // 128              # 16 k-chunks
MSB = S // 512             # 4 m-superblocks
NB = S // 128              # 16 n-blocks
VST = 130                  # v_sb per-block stride: 64 v_a | 1 | 64 v_b | 1
INV_SQRT_DIMK = 1.0 / math.sqrt(float(DIMK))


# ---------------------------------------------------------------------------
# TileContext tail-drain patch: the bundled neuronxcc walrus rejects
# instructions carrying more than ~2 sync waits ("Too many sync wait
# commands"). Spread the kernel-tail waits over single-wait nops.
def _patched_drain_and_barrier(self, tick_clock, wait_clock):
    nc = self.nc
    collector = nc.sync.nop(nofuse=True)
    wait_clock.add_sem_waits(
        collector.ins, ScopedClock({None: tick_clock.global_clock})
    )
    si = collector.ins.sync_info
    waits = list(si.on_wait) if si is not None and si.on_wait else []
    if waits:
        collector.ins.sync_info = mybir.SyncInfo(
            on_wait=[waits[0]], on_update=list(si.on_update or [])
        )
        for w in waits[1:]:
            extra = nc.sync.nop(nofuse=True)
            extra.ins.sync_info = mybir.SyncInfo(on_wait=[w], on_update=[])
    nc.sync.drain()
    nc.all_engine_barrier()
    assert self.sems is not None
    popped = nc._tile_sem_poison_stack.pop()
    assert popped is self._sem_poison
    nc.clear_and_free_semaphores(list(self.sems.allocated().values()))
    nc.all_engine_barrier()


tile.TileContext._drain_and_barrier = _patched_drain_and_barrier


_MAXW = 1
_NOPID = [0]


def split_excess_waits(nc):
    """Walrus here encodes at most ~1-2 sync waits per instruction; move the
    excess onto preceding same-engine nops (engine order preserves timing)."""
    for f in nc.m.functions:
        for bb in f.blocks:
            out_list = []
            changed = False
            for inst in bb.instructions:
                si = getattr(inst, "sync_info", None)
                waits = list(si.on_wait) if si is not None and si.on_wait else []
                if len(waits) > _MAXW:
                    changed = True
                    for w in waits[:-_MAXW]:
                        _NOPID[0] += 1
                        nop = mybir.InstNoOp(
                            name=f"waitnop-{_NOPID[0]}", ins=[], outs=[],
                            engine=inst.engine,
                        )
                        nop.sync_info = mybir.SyncInfo(on_wait=[w], on_update=[])
                        out_list.append(nop)
                    inst.sync_info = mybir.SyncInfo(
                        on_wait=waits[-_MAXW:], on_update=list(si.on_update or [])
                    )
                out_list.append(inst)
            if changed:
                bb.instructions[:] = out_list
# ---------------------------------------------------------------------------


def build_bass():
    nc = bass.Bass()
    xqT = nc.dram_tensor("xqT", [D, S], BF16, kind="ExternalInput")
    xkT = nc.dram_tensor("xkT", [D, S], BF16, kind="ExternalInput")
    xvT = nc.dram_tensor("xvT", [D, S], BF16, kind="ExternalInput")
    wq = nc.dram_tensor("wq", [D, CPC], BF16, kind="ExternalInput")
    wk = nc.dram_tensor("wk", [D, 128], BF16, kind="ExternalInput")
    wv = nc.dram_tensor("wv", [D, 128], BF16, kind="ExternalInput")
    wo = nc.dram_tensor("wo", [CPC, D], BF16, kind="ExternalInput")
    bq = nc.dram_tensor("bq", [CPC, 1], F32, kind="ExternalInput")
    bk = nc.dram_tensor("bk", [128, 1], F32, kind="ExternalInput")
    trimask = nc.dram_tensor("trimask", [128, 128], BF16, kind="ExternalInput")
    ident = nc.dram_tensor("ident", [128, 128], BF16, kind="ExternalInput")
    out = nc.dram_tensor("out", [S, D], BF16, kind="ExternalOutput")

    from contextlib import ExitStack
    with tile.TileContext(nc) as tc, ExitStack() as ctx:
        build_body(ctx, tc, xqT, xkT, xvT, wq, wk, wv, wo, bq, bk,
                   trimask, ident, out)
    split_excess_waits(nc)
    return nc


def build_body(ctx, tc, xqT, xkT, xvT, wq, wk, wv, wo, bq, bk,
               trimask, ident, out):
    nc = tc.nc
    Exp = mybir.ActivationFunctionType.Exp
    Ident = mybir.ActivationFunctionType.Identity
    Copy = mybir.ActivationFunctionType.Copy

    const = ctx.enter_context(tc.tile_pool(name="const", bufs=1))
    wq_sb = const.tile([128, KC * CPC], BF16, tag="wq")      # [128, 8192]
    wk_sb = const.tile([128, KC * 128], BF16, tag="wk")      # [128, 2048]
    wv_sb = const.tile([128, KC * 128], BF16, tag="wv")      # [128, 2048]
    wo_sb = const.tile([128, 4 * D], BF16, tag="wo")         # [128, 8192]
    kT_sb = const.tile([128, S], BF16, tag="kT")             # [128, 2048]
    v_sb = const.tile([128, NB * VST], BF16, tag="v")        # [128, 2080]
    qT_sb = const.tile([128, 4 * S], BF16, tag="qT")         # [128, 8192]
    bq_sb = const.tile([128, 4], F32, tag="bq")
    bk_sb = const.tile([128, 1], F32, tag="bk")
    mask_sb = const.tile([128, 128], BF16, tag="mask")
    ident_sb = const.tile([128, 128], BF16, tag="ident")
    ones_row = const.tile([1, 64], BF16, tag="ones_row")

    # Weight / bias / mask loads (sync queue; x streams use other queues)
    nc.sync.dma_start(
        wq_sb[:].rearrange("p (kc c) -> p kc c", kc=KC),
        wq.rearrange("(kc p) c -> p kc c", p=128),
    )
    nc.sync.dma_start(
        wk_sb[:].rearrange("p (kc c) -> p kc c", kc=KC),
        wk.rearrange("(kc p) c -> p kc c", p=128),
    )
    nc.sync.dma_start(
        wv_sb[:].rearrange("p (kc c) -> p kc c", kc=KC),
        wv.rearrange("(kc p) c -> p kc c", p=128),
    )
    nc.sync.dma_start(
        wo_sb[:].rearrange("p (cb d) -> p cb d", cb=4),
        wo.rearrange("(cb p) d -> p cb d", p=128),
    )
    nc.sync.dma_start(
        bq_sb[:].rearrange("p (cb o) -> p cb o", cb=4),
        bq.rearrange("(cb p) o -> p cb o", p=128),
    )
    nc.sync.dma_start(bk_sb[:], bk[:])
    nc.sync.dma_start(mask_sb[:], trimask[:])
    nc.sync.dma_start(ident_sb[:], ident[:])
    nc.vector.memset(v_sb[:], 1.0)   # ones columns at 64/129 of each block
    nc.vector.memset(ones_row[:], 1.0)

    # ---------------- Phase 1-3: projections ----------------
    with tc.tile_pool(name="proj_psum", bufs=4, space="PSUM") as proj_psum, \
         tc.tile_pool(name="tp_psum", bufs=2, space="PSUM") as tp_psum, \
         tc.tile_pool(name="xin", bufs=3) as xin_pool, \
         tc.tile_pool(name="xvin", bufs=3) as xvin_pool, \
         tc.tile_pool(name="vt", bufs=2) as vt_pool:

        # K projection: kT[c=128, n] accumulated over k-chunks, bias bk.
        for nsb in range(4):
            ps = proj_psum.tile([128, 512], F32, tag="ps")
            for kc in range(KC):
                xk_t = xin_pool.tile([128, 512], BF16, tag="xk")
                nc.gpsimd.dma_start(
                    xk_t[:], xkT[kc * 128:(kc + 1) * 128, nsb * 512:(nsb + 1) * 512]
                )
                nc.tensor.matmul(
                    ps[:], wk_sb[:, kc * 128:(kc + 1) * 128], xk_t[:],
                    start=(kc == 0), stop=(kc == KC - 1),
                )
            nc.scalar.activation(
                kT_sb[:, nsb * 512:(nsb + 1) * 512], ps[:], Ident, bias=bk_sb[:]
            )

        # V projection: transposed (vT[c, n]) with wv stationary, then PE
        # transposes into v natural layout with interleaved ones columns.
        for nsb in range(4):
            ps = proj_psum.tile([128, 512], F32, tag="ps")
            for kc in range(KC):
                xv_t = xvin_pool.tile([128, 512], BF16, tag="xv")
                nc.sync.dma_start(
                    xv_t[:], xvT[kc * 128:(kc + 1) * 128, nsb * 512:(nsb + 1) * 512]
                )
                nc.tensor.matmul(
                    ps[:], wv_sb[:, kc * 128:(kc + 1) * 128], xv_t[:],
                    start=(kc == 0), stop=(kc == KC - 1),
                )
            vT_t = vt_pool.tile([128, 512], BF16, tag="vt")
            nc.vector.tensor_copy(vT_t[:], ps[:])
            for t in range(4):
                j = nsb * 4 + t
                tp = tp_psum.tile([128, 128], BF16, tag="tp")
                nc.tensor.transpose(tp[:], vT_t[:, t * 128:(t + 1) * 128],
                                    ident_sb[:])
                nc.vector.tensor_copy(v_sb[:, j * VST: j * VST + 64],
                                      tp[:, 0:64])
                nc.vector.tensor_copy(v_sb[:, j * VST + 65: j * VST + 129],
                                      tp[:, 64:128])

        # Q projection: qT[c, m], layout [msb][cb][512], bias bq.
        for msb in range(MSB):
            pss = [proj_psum.tile([128, 512], F32, tag="ps", name=f"psq{cb}")
                   for cb in range(4)]
            for kc in range(KC):
                xq_t = xin_pool.tile([128, 512], BF16, tag="xk")
                nc.scalar.dma_start(
                    xq_t[:], xqT[kc * 128:(kc + 1) * 128, msb * 512:(msb + 1) * 512]
                )
                for cb in range(4):
                    nc.tensor.matmul(
                        pss[cb][:],
                        wq_sb[:, kc * CPC + cb * 128: kc * CPC + (cb + 1) * 128],
                        xq_t[:],
                        start=(kc == 0), stop=(kc == KC - 1),
                    )
            for cb in range(4):
                nc.scalar.activation(
                    qT_sb[:, msb * 2048 + cb * 512: msb * 2048 + (cb + 1) * 512],
                    pss[cb][:], Ident, bias=bq_sb[:, cb:cb + 1],
                )

    # ---------------- Phase 4: attention + output projection ----------------
    with tc.tile_pool(name="lt_psum", bufs=2, space="PSUM") as lt_psum, \
         tc.tile_pool(name="acc_psum", bufs=1, space="PSUM") as acc_psum, \
         tc.tile_pool(name="aux_psum", bufs=2, space="PSUM") as aux_psum, \
         tc.tile_pool(name="pt", bufs=3) as pt_pool, \
         tc.tile_pool(name="outT", bufs=2) as outT_pool, \
         tc.tile_pool(name="nrm", bufs=2) as nrm_pool, \
         tc.tile_pool(name="osb", bufs=3) as out_pool:

        def emit_wo(msb, outT_t):
            for mb in range(4):
                for db in range(4):
                    pso = aux_psum.tile([128, 512], F32, tag="aux")
                    for cb in range(4):
                        nc.tensor.matmul(
                            pso[:],
                            outT_t[:, cb * 512 + mb * 128: cb * 512 + (mb + 1) * 128],
                            wo_sb[:, cb * D + db * 512: cb * D + (db + 1) * 512],
                            start=(cb == 0), stop=(cb == 3),
                        )
                    o_t = out_pool.tile([128, 512], BF16, tag="osb")
                    nc.vector.tensor_copy(o_t[:], pso[:])
                    nc.gpsimd.dma_start(
                        out[msb * 512 + mb * 128: msb * 512 + (mb + 1) * 128,
                            db * 512:(db + 1) * 512],
                        o_t[:],
                    )

        outT_prev = None
        for msb in range(MSB):
            outT_t = outT_pool.tile([128, 2048], BF16, tag="outT")
            for p in range(4):
                acc_a = acc_psum.tile([128, 512], F32, tag="acca")
                acc_b = acc_psum.tile([128, 512], F32, tag="accb")
                njb = 4 * msb + 4
                qbase = msb * 2048 + p * 512
                for j in range(njb):
                    if j < 4 * msb:
                        moff, W = 0, 512
                    else:
                        t = j - 4 * msb
                        moff, W = 128 * t, 512 - 128 * t
                    first = (j == 0)
                    last = (j == njb - 1)
                    qlo = qT_sb[0:64, qbase + moff: qbase + moff + W]
                    qhi = qT_sb[64:128, qbase + moff: qbase + moff + W]
                    lt = lt_psum.tile([128, 1024], F32, tag="lt")
                    nc.tensor.matmul(
                        lt[:, 0:W],
                        kT_sb[0:64, j * 128:(j + 1) * 128], qlo,
                        start=True, stop=True, tile_position=(0, 0),
                    )
                    nc.tensor.matmul(
                        lt[:, 512:512 + W],
                        kT_sb[64:128, j * 128:(j + 1) * 128], qhi,
                        start=True, stop=True, tile_position=(64, 0),
                    )
                    pt = pt_pool.tile([128, 1024], BF16, tag="pt")
                    nc.scalar.activation(pt[:], lt[:], Exp, scale=INV_SQRT_DIMK)
                    if j >= 4 * msb:  # diagonal: mask the leading triangle
                        nc.gpsimd.tensor_mul(pt[:, 0:128], pt[:, 0:128],
                                             mask_sb[:])
                        nc.gpsimd.tensor_mul(pt[:, 512:640], pt[:, 512:640],
                                             mask_sb[:])
                    # attn @ v with the denominator folded in (ones col at 64)
                    nc.tensor.matmul(
                        acc_a[0:65, moff:moff + W],
                        v_sb[:, j * VST: j * VST + 65], pt[:, 0:W],
                        start=first, stop=last,
                    )
                    nc.tensor.matmul(
                        acc_b[0:65, moff:moff + W],
                        v_sb[:, j * VST + 65: j * VST + 130], pt[:, 512:512 + W],
                        start=first, stop=last,
                    )
                # normalize: reciprocal rows (bf16 out is plenty at our
                # tolerance), PE outer-product broadcast, fused multiply.
                ra16 = nrm_pool.tile([1, 512], BF16, tag="ra16",
                                     name=f"ra16{msb}{p}")
                rb16 = nrm_pool.tile([1, 512], BF16, tag="rb16",
                                     name=f"rb16{msb}{p}")
                with nc.allow_low_precision(reason="softmax denom recip"):
                    nc.vector.reciprocal(ra16[:], acc_a[64:65, :])
                    nc.vector.reciprocal(rb16[:], acc_b[64:65, :])
                bps = aux_psum.tile([128, 512], F32, tag="aux")
                nc.tensor.matmul(
                    bps[0:64, :], ones_row[:], ra16[:],
                    start=True, stop=True, tile_position=(0, 0),
                )
                nc.tensor.matmul(
                    bps[64:128, :], ones_row[:], rb16[:],
                    start=True, stop=True, tile_position=(0, 64),
                )
                bc = nrm_pool.tile([128, 512], BF16, tag="bc")
                nc.vector.tensor_copy(bc[:], bps[:])
                nc.vector.tensor_mul(
                    outT_t[0:64, p * 512:(p + 1) * 512], acc_a[0:64, :],
                    bc[0:64, :],
                )
                nc.vector.tensor_mul(
                    outT_t[64:128, p * 512:(p + 1) * 512], acc_b[0:64, :],
                    bc[64:128, :],
                )
                if p == 0 and outT_prev is not None:
                    emit_wo(msb - 1, outT_prev)
            outT_prev = outT_t
        emit_wo(MSB - 1, outT_prev)


_NC_CACHE = {}


def get_nc():
    if "nc" not in _NC_CACHE:
        _NC_CACHE["nc"] = build_bass()
    return _NC_CACHE["nc"]


def kernel(inputs_q, inputs_k, inputs_v, Wq, bq, Wk, bk, Wv, bv, Wo, bo):
    inputs_q = np.asarray(inputs_q, np.float32)
    inputs_k = np.asarray(inputs_k, np.float32)
    inputs_v = np.asarray(inputs_v, np.float32)
    Wq = np.asarray(Wq, np.float32)
    Wk = np.asarray(Wk, np.float32)
    Wv = np.asarray(Wv, np.float32)
    Wo = np.asarray(Wo, np.float32)
    bq = np.asarray(bq, np.float32)
    bk = np.asarray(bk, np.float32)
    bv = np.asarray(bv, np.float32)
    bo = np.asarray(bo, np.float32)

    nc = get_nc()
    trimask = np.triu(np.ones((128, 128), np.float32)).astype(NPBF16)
    identity = np.eye(128, dtype=np.float32).astype(NPBF16)

    xT = {}
    for b in range(B):
        xT[("q", b)] = np.ascontiguousarray(inputs_q[b].T).astype(NPBF16)
        xT[("k", b)] = np.ascontiguousarray(inputs_k[b].T).astype(NPBF16)
        xT[("v", b)] = np.ascontiguousarray(inputs_v[b].T).astype(NPBF16)

    in_maps = []
    for c in range(8):
        b = c // 4
        g0 = 2 * (c % 4)
        g1 = g0 + 1
        # pair-major channel permutation: (head p of g0, head p of g1), p=0..3
        perm = []
        for p in range(HPG):
            perm.extend(range(256 * g0 + 64 * p, 256 * g0 + 64 * p + 64))
            perm.extend(range(256 * g1 + 64 * p, 256 * g1 + 64 * p + 64))
        perm = np.array(perm)
        in_maps.append({
            "xqT": xT[("q", b)],
            "xkT": xT[("k", b)],
            "xvT": xT[("v", b)],
            "wq": Wq[:, perm].astype(NPBF16),
            "wk": Wk[:, 64 * g0: 64 * g0 + 128].astype(NPBF16),
            "wv": Wv[:, 64 * g0: 64 * g0 + 128].astype(NPBF16),
            "wo": Wo[perm, :].astype(NPBF16),
            "bq": np.ascontiguousarray(bq[perm].reshape(CPC, 1)),
            "bk": np.ascontiguousarray(bk[64 * g0: 64 * g0 + 128].reshape(128, 1)),
            "trimask": trimask,
            "ident": identity,
        })

    res = run_bass_kernel_spmd(nc, in_maps, list(range(8)))

    # bv passes through (attention rows sum to 1): out += bv_expand @ Wo + bo
    bv_expand = np.repeat(bv.reshape(NKV, 1, HD), HPG, axis=1).reshape(D)
    corr = (bv_expand.astype(np.float64) @ Wo.astype(np.float64)) + bo

    outp = np.zeros((B, S, D), np.float32)
    for c in range(8):
        outp[c // 4] += res.results[c]["out"].astype(np.float32)
    outp += corr.astype(np.float32)
    return outp
